# revision 1
# baseline (speedup 1.0000x reference)
"""MultiHeadAttention Trainium2 Bass kernel (8-core SPMD).

Problem: B=2, S=2048, DIM=1024, H=16 heads (dh=64), fp32 reference.
Sharding: core c handles batch b = c//4 and 4 heads ho = 4*(c%4)..+4
(data-parallel over batch x tensor-parallel over heads). Each core:
  qhT/khT = W{q,k}.T-slice @ x.T + b   -> [256, 2048] bf16 (head-dim major)
  vh_aug  = x @ Wv.T-slice + bv (+ones col per head) per k-tile
  scores^T = kh @ qh.T (per head, K=64 row-packed pairs)
  P^T = keepmask * exp(SCALE * scores^T)                (ACT + DVE)
  out^T[65|128, q] = [vh | ones].T @ P^T                (PV + row-sums fused)
  normalize by 1/sums (per-qt batched reciprocal), y^T = Wo.T-slice.T @ O^T
Host gathers: y[b] = sum over 4 cores of y^T_partial.T, + bo.

All bulk tensors are pre-tiled on the host into the exact [128, 512]/[128,
1024] tiles the kernel consumes, so every DMA is one fully contiguous
128/256KB transfer (strided 1KB-row DMAs measured at only ~190GB/s
aggregate -- packet-rate-bound).
"""

import os
import sys

sys.path.insert(0, "/opt/trn_rl_repo")
os.environ.setdefault("MYCRO_LOCAL_CACHE", "1")

import numpy as np

import concourse.bass as bass
import concourse.bacc as bacc
import concourse.tile as tile
from concourse import mybir
from concourse import bass_utils

F32 = mybir.dt.float32
BF16 = mybir.dt.bfloat16
NP_BF16 = mybir.dt.np(BF16)

B, S, DIM = 2, 2048, 1024
H = 16
DH = 64
SCALE = 1.0 / (DIM ** 0.5)
N_CORES = 8
HPC = 4          # heads per core
QT = S // 512    # 4 q-chunks of 512
KT = S // 128    # 16 k-tiles of 128
CT = DIM // 128  # 8 contraction tiles for projections

# vh_aug per-kt layout: per pair p (2 local pairs):
#   A block: [vh_A(64) | ones(1)]                 at cols p*193 + [0, 65)
#   B block: [zeros(32) | ones(1) | zeros(31) | vh_B(64)] at cols p*193 + [65, 193)
#   (B ones at col +97 so B sums land on psum partition 32 -- DVE start
#   partitions must be in {0, 32, 64, 96})
VHA_W = 386


def build_nc():
    # Bacc (not plain Bass): its compile() pipeline splits multi-semaphore
    # waits into event-semaphore chains -- walrus codegen allows only ONE
    # sync wait per compute instruction on TRN2.
    nc = bacc.Bacc("TRN2", target_bir_lowering=False)

    xq_d = nc.declare_dram_parameter("xq", [QT, CT, 128, 512], BF16, isOutput=False)
    xk_d = nc.declare_dram_parameter("xk", [QT, CT, 128, 512], BF16, isOutput=False)
    xv_d = nc.declare_dram_parameter("xv", [QT, CT, 128, 512], BF16, isOutput=False)
    wq_d = nc.declare_dram_parameter("wq", [CT, 128, 256], BF16, isOutput=False)
    wk_d = nc.declare_dram_parameter("wk", [CT, 128, 256], BF16, isOutput=False)
    wv_d = nc.declare_dram_parameter("wv", [CT, 128, 256], BF16, isOutput=False)
    wo_d = nc.declare_dram_parameter("wo", [2, 128, 1024], BF16, isOutput=False)
    bq_d = nc.declare_dram_parameter("bq2", [2, 128, 1], F32, isOutput=False)
    bk_d = nc.declare_dram_parameter("bk2", [2, 128, 1], F32, isOutput=False)
    bvb_d = nc.declare_dram_parameter("bvb", [128, 256], BF16, isOutput=False)
    mk_d = nc.declare_dram_parameter("mk", [KT, QT, 128, 512], BF16, isOutput=False)
    yt_d = nc.declare_dram_parameter("yt", [8, 2, 128, 1024], BF16, isOutput=True)
    rscr_d = nc.dram_tensor("rscr", [HPC, S], F32)

    with tile.TileContext(nc) as tc:
        with tc.tile_pool(name="persist", bufs=1) as singles:
            # ---- biases + weights FIRST on the DMA queues (a late bias
            # gates the first DVE cast and stalls the whole machine) ----
            bq_sb, bk_sb = [], []
            for m in range(2):
                tq = singles.tile([128, 1], F32, tag=f"bq{m}", name=f"bq{m}")
                nc.sync.dma_start(out=tq, in_=bq_d[m])
                bq_sb.append(tq)
                tk = singles.tile([128, 1], F32, tag=f"bk{m}", name=f"bk{m}")
                nc.sync.dma_start(out=tk, in_=bk_d[m])
                bk_sb.append(tk)
            bvb_sb = singles.tile([128, 256], BF16, tag="bvb")
            nc.sync.dma_start(out=bvb_sb, in_=bvb_d[:, :])

            def load_rows(dram, n_tiles, width, tag):
                tiles = []
                for c in range(n_tiles):
                    t = singles.tile([128, width], BF16, tag=f"{tag}{c}", name=f"{tag}{c}")
                    nc.sync.dma_start(out=t, in_=dram[c])
                    tiles.append(t)
                return tiles

            wq_sb = load_rows(wq_d, CT, 256, "wq")
            wk_sb = load_rows(wk_d, CT, 256, "wk")
            wv_sb = load_rows(wv_d, CT, 256, "wv")
            wo_sb = load_rows(wo_d, 2, DIM, "wo")

            def load_x(dram, tag):
                # chunk-major: all 8 c-tiles of a column chunk arrive together
                tiles = [[None] * QT for _ in range(CT)]
                for n in range(QT):
                    for c in range(CT):
                        t = singles.tile([128, 512], BF16,
                                         tag=f"{tag}{c}_{n}", name=f"{tag}{c}_{n}")
                        nc.sync.dma_start(out=t, in_=dram[n, c])
                        tiles[c][n] = t
                return tiles

            xv_sb = load_x(xv_d, "xv")
            xk_sb = load_x(xk_d, "xk")
            xq_sb = load_x(xq_d, "xq")

            # ---- persistent intermediates ----
            qhT = [[singles.tile([128, 512], BF16, tag=f"qhT{m}_{n}",
                                 name=f"qhT{m}_{n}") for n in range(QT)]
                   for m in range(2)]
            khT = [[singles.tile([128, 512], BF16, tag=f"khT{m}_{n}",
                                 name=f"khT{m}_{n}") for n in range(QT)]
                   for m in range(2)]
            OT = [singles.tile([128, S], BF16, tag=f"OT{m}", name=f"OT{m}") for m in range(2)]
            vha = [singles.tile([128, VHA_W], BF16, tag=f"vha{kt}",
                                name=f"vha{kt}") for kt in range(KT)]
            sums_stage = singles.tile([128, 2, S], F32, tag="sums_stage")

            for kt in range(KT):
                for p in range(2):
                    base = p * 193
                    nc.gpsimd.memset(vha[kt][:, base + 64:base + 65], 1.0)
                    nc.gpsimd.memset(vha[kt][:, base + 97:base + 98], 1.0)
                    nc.gpsimd.memset(vha[kt][:, base + 65:base + 97], 0.0)
                    nc.gpsimd.memset(vha[kt][:, base + 98:base + 129], 0.0)

            # ---- projections (own scoped psum pool, v1-style) ----
            with tc.tile_pool(name="pjp", bufs=2, space="PSUM") as pj:
                # PE warmup to open the HAM clock gate while DMAs land
                warm = singles.tile([128, 512], BF16, tag="warm")
                nc.gpsimd.memset(warm[:, :], 0.0)
                wps = pj.tile([128, 512], F32, tag="pqk", name="wps")
                for i in range(24):
                    nc.tensor.matmul(
                        wps, warm[:, 0:128], warm[:, :],
                        start=True, stop=True)

                for kt in range(KT):
                    ps = pj.tile([128, 256], F32, tag="pv", name="psv")
                    for c in range(CT):
                        nc.tensor.matmul(
                            ps,
                            xv_sb[c][kt // 4][:, (kt % 4) * 128:(kt % 4 + 1) * 128],
                            wv_sb[c],
                            start=(c == 0),
                            stop=(c == CT - 1),
                        )
                    for h in range(HPC):
                        p, is_b = h // 2, h % 2
                        col = p * 193 + (129 if is_b else 0)
                        nc.vector.tensor_tensor(
                            out=vha[kt][:, col:col + 64],
                            in0=ps[:, h * 64:(h + 1) * 64],
                            in1=bvb_sb[:, h * 64:(h + 1) * 64],
                            op=mybir.AluOpType.add,
                        )
                for x_sb, w_sb, b_sb, dst in (
                    (xk_sb, wk_sb, bk_sb, khT),
                    (xq_sb, wq_sb, bq_sb, qhT),
                ):
                    for m in range(2):
                        for n in range(QT):
                            ps = pj.tile([128, 512], F32, tag="pqk", name="psqk")
                            for c in range(CT):
                                nc.tensor.matmul(
                                    ps,
                                    w_sb[c][:, m * 128:(m + 1) * 128],
                                    x_sb[c][n],
                                    start=(c == 0),
                                    stop=(c == CT - 1),
                                )
                            bb = b_sb[m][:, 0:1]
                            bb_bc = bass.AP(
                                tensor=bb.tensor, offset=bb.offset,
                                ap=[list(bb.ap[0]), [0, 512]])
                            nc.vector.tensor_tensor(
                                out=dst[m][n],
                                in0=ps,
                                in1=bb_bc,
                                op=mybir.AluOpType.add,
                            )

            # ---- attention: v1 structure (pair-merged, shared mask) ----
            with tc.tile_pool(name="scp", bufs=2, space="PSUM") as scp, \
                 tc.tile_pool(name="pvp", bufs=2, space="PSUM") as pvp:
                for qt in range(QT):
                    po = [pvp.tile([128, 1024], F32, tag="po", name="po")
                          for _ in range(2)]
                    for kt in range(KT):
                        mt = singles.tile([128, 512], BF16, tag="mask",
                                          name="mask", bufs=6)
                        nc.sync.dma_start(out=mt, in_=mk_d[kt, qt])
                        m_ap = mt[:, :]
                        mbc = bass.AP(
                            tensor=m_ap.tensor,
                            offset=m_ap.offset,
                            ap=[list(m_ap.ap[0]), [0, 2], list(m_ap.ap[1])],
                        )
                        for p in range(2):
                            ps = scp.tile([128, 1024], F32, tag="sc", name="ps")
                            for ab in range(2):
                                nc.tensor.matmul(
                                    ps[:, ab * 512:(ab + 1) * 512],
                                    khT[p][kt // 4][ab * 64:(ab + 1) * 64,
                                                    (kt % 4) * 128:(kt % 4 + 1) * 128],
                                    qhT[p][qt][ab * 64:(ab + 1) * 64, :],
                                    start=True,
                                    stop=True,
                                )
                            pt = singles.tile([128, 1024], BF16, tag="pt",
                                              name="pt", bufs=4)
                            nc.scalar.activation(
                                out=pt, in_=ps,
                                func=mybir.ActivationFunctionType.Exp,
                                scale=float(SCALE),
                            )
                            nc.vector.tensor_tensor(
                                out=pt, in0=pt, in1=mbc,
                                op=mybir.AluOpType.mult,
                            )
                            base = p * 193
                            nc.tensor.matmul(
                                po[p][0:65, 0:512],
                                vha[kt][:, base:base + 65],
                                pt[:, 0:512],
                                start=(kt == 0), stop=(kt == KT - 1),
                            )
                            nc.tensor.matmul(
                                po[p][:, 512:1024],
                                vha[kt][:, base + 65:base + 193],
                                pt[:, 512:1024],
                                start=(kt == 0), stop=(kt == KT - 1),
                            )
                    for p in range(2):
                        qsl = slice(qt * 512, (qt + 1) * 512)
                        nc.vector.tensor_copy(
                            out=OT[p][0:64, qsl], in_=po[p][0:64, 0:512])
                        nc.vector.tensor_copy(
                            out=OT[p][64:128, qsl], in_=po[p][64:128, 512:1024])
                        nc.vector.tensor_copy(
                            out=sums_stage[64:65, p, qsl],
                            in_=po[p][64:65, 0:512])
                        nc.vector.tensor_copy(
                            out=sums_stage[32:33, p, qsl],
                            in_=po[p][32:33, 512:1024])

                # ---- batched normalization ----
                recin = singles.tile([128, 64], F32, tag="recin")
                for h in range(HPC):
                    p, is_b = h // 2, h % 2
                    row = 32 if is_b else 64
                    nc.sync.dma_start(
                        out=recin[:, h * 16:(h + 1) * 16],
                        in_=sums_stage[row:row + 1, p, :])
                recout = singles.tile([128, 64], F32, tag="recout")
                nc.vector.reciprocal(out=recout, in_=recin)
                for h in range(HPC):
                    nc.sync.dma_start(
                        out=rscr_d[h:h + 1, :],
                        in_=recout[:, h * 16:(h + 1) * 16])
                for p in range(2):
                    rbc = singles.tile([128, S], F32, tag=f"rbc{p}", name=f"rbc{p}")
                    for ab in range(2):
                        srow = rscr_d[2 * p + ab:2 * p + ab + 1, :]
                        src_bc = bass.AP(
                            tensor=srow.tensor,
                            offset=srow.offset,
                            ap=[[0, 64], list(srow.ap[-1])],
                        )
                        nc.sync.dma_start(
                            out=rbc[ab * 64:(ab + 1) * 64, :], in_=src_bc)
                    nc.vector.tensor_tensor(
                        out=OT[p], in0=OT[p], in1=rbc,
                        op=mybir.AluOpType.mult)

            # ---- output projection ----
            with tc.tile_pool(name="oyp", bufs=4, space="PSUM") as oyp:
                for ot in range(8):
                    for half in range(2):
                        ps = oyp.tile([128, 1024], F32, tag="py", name="psy")
                        for p in range(2):
                            for n in range(2):
                                nc.tensor.matmul(
                                    ps[:, n * 512:(n + 1) * 512],
                                    wo_sb[p][:, ot * 128:(ot + 1) * 128],
                                    OT[p][:, (half * 2 + n) * 512:
                                          (half * 2 + n + 1) * 512],
                                    start=(p == 0),
                                    stop=(p == 1),
                                )
                        yt = singles.tile([128, 1024], BF16, tag="yt",
                                          name="yt", bufs=4)
                        nc.scalar.copy(out=yt, in_=ps)
                        nc.sync.dma_start(out=yt_d[ot, half], in_=yt)
    nc.compile()
    return nc


_NC_CACHE = None


def get_nc():
    global _NC_CACHE
    if _NC_CACHE is None:
        _NC_CACHE = build_nc()
    return _NC_CACHE


def _tile_x(xT):
    # [1024, 2048] -> [QT, CT, 128, 512]
    return np.ascontiguousarray(
        xT.reshape(CT, 128, QT, 512).transpose(2, 0, 1, 3))


def prep_in_maps(q, k, v, mask, Wq, bq, Wk, bk, Wv, bv, Wo, bo):
    q = np.asarray(q, np.float32)
    k = np.asarray(k, np.float32)
    v = np.asarray(v, np.float32)
    mask = np.asarray(mask)
    WqT = np.asarray(Wq, np.float32).T
    WkT = np.asarray(Wk, np.float32).T
    WvT = np.asarray(Wv, np.float32).T
    WoT = np.asarray(Wo, np.float32).T
    bq = np.asarray(bq, np.float32)
    bk = np.asarray(bk, np.float32)
    bv = np.asarray(bv, np.float32)

    xT = {}
    keepT = {}
    for b in range(B):
        xT[b] = (
            _tile_x(np.ascontiguousarray(q[b].T).astype(NP_BF16)),
            _tile_x(np.ascontiguousarray(k[b].T).astype(NP_BF16)),
            _tile_x(np.ascontiguousarray(v[b].T).astype(NP_BF16)),
        )
        mt = np.ascontiguousarray((~mask[b, 0]).T.astype(np.float32)).astype(NP_BF16)
        keepT[b] = np.ascontiguousarray(
            mt.reshape(KT, 128, QT, 512).transpose(0, 2, 1, 3))

    in_maps = []
    for c in range(N_CORES):
        b = c // 4
        ho = c % 4
        dsl = slice(ho * 256, ho * 256 + 256)
        xq, xk, xv = xT[b]
        in_maps.append({
            "xq": xq,
            "xk": xk,
            "xv": xv,
            "wq": np.ascontiguousarray(WqT[:, dsl]).astype(NP_BF16).reshape(CT, 128, 256),
            "wk": np.ascontiguousarray(WkT[:, dsl]).astype(NP_BF16).reshape(CT, 128, 256),
            "wv": np.ascontiguousarray(WvT[:, dsl]).astype(NP_BF16).reshape(CT, 128, 256),
            "wo": np.ascontiguousarray(WoT[dsl, :]).astype(NP_BF16).reshape(2, 128, 1024),
            "bq2": np.ascontiguousarray(bq[dsl]).reshape(2, 128, 1).astype(np.float32),
            "bk2": np.ascontiguousarray(bk[dsl]).reshape(2, 128, 1).astype(np.float32),
            "bvb": np.ascontiguousarray(
                np.broadcast_to(bv[dsl], (128, 256))).astype(NP_BF16),
            "mk": keepT[b],
        })
    return in_maps


def gather_output(results, bo):
    bo = np.asarray(bo, np.float32)
    y = np.zeros((B, S, DIM), np.float32)
    for c in range(N_CORES):
        yt = np.asarray(results[c]["yt"], np.float32)  # [8, 2, 128, 1024]
        yT = yt.transpose(0, 2, 1, 3).reshape(DIM, S)
        y[c // 4] += yT.T
    y += bo[None, None, :]
    return y


def kernel(**inputs):
    nc = get_nc()
    in_maps = prep_in_maps(**{k_: inputs[k_] for k_ in (
        "q", "k", "v", "mask", "Wq", "bq", "Wk", "bk", "Wv", "bv", "Wo", "bo")})
    res = bass_utils.run_bass_kernel_spmd(nc, in_maps, list(range(N_CORES)))
    return gather_output(res.results, inputs["bo"])



# revision 4
# speedup vs baseline: 1.0498x; 1.0498x over previous
"""MultiHeadAttention Trainium2 Bass kernel (8-core SPMD), v2.

Problem: B=2, S=2048, DIM=1024, H=16 heads (dh=64), fp32 reference.
Sharding: core c handles batch b = c//4 and 4 heads ho = 4*(c%4)..+4
(data-parallel over batch x tensor-parallel over heads).

v2 structural changes over v1 (300us -> target ~190us):
  - Wide-row DMAs: x tensors as [CT, 128, 2048] (4KB/partition rows) and
    mask as [QT, 4, 128, 2048] kt-interleaved groups; 1KB rows measured
    packet-rate-bound at ~200GB/s aggregate.
  - DMA priority order: weights -> xk -> xv -> xq(n0 cols) -> first masks
    -> xq(rest). Attention starts as soon as kh (all), vha (all) and
    qh[n=0] exist (~30us instead of ~100us).
  - K-proj runs c-outer (arrival-paced, 8 psum banks); V-proj c-outer after
    it; Q-proj n=0 in the attention scp pool; Q-proj n=1..3 emitted inside
    the preceding qt's kt-loop (PE slack) using scp-pool tiles.
  - Per-qt softmax normalization fully overlapped with the next qt's
    attention (v1 did it all at the end: 20us serial bubble).
  - Out-projection PSUM->SBUF casts split across DVE+ACT at the tail.

Per-core engine floors: ACT exp 128 x (352+1024)/1.2 = 147us (bottleneck),
PE ~140us, DVE ~110us, DMA ~25MB.
"""

import os
import sys

sys.path.insert(0, "/opt/trn_rl_repo")
os.environ.setdefault("MYCRO_LOCAL_CACHE", "1")

import numpy as np

import concourse.bass as bass
import concourse.bacc as bacc
import concourse.tile as tile
from concourse import mybir
from concourse import bass_utils

F32 = mybir.dt.float32
BF16 = mybir.dt.bfloat16
NP_BF16 = mybir.dt.np(BF16)

B, S, DIM = 2, 2048, 1024
H = 16
DH = 64
SCALE = 1.0 / (DIM ** 0.5)
N_CORES = 8
HPC = 4          # heads per core
QT = S // 512    # 4 q-chunks of 512
KT = S // 128    # 16 k-tiles of 128
CT = DIM // 128  # 8 contraction tiles for projections
KG = 4           # kt-tiles per mask group

# vh_aug per-kt layout: per pair p (2 local pairs):
#   A block: [vh_A(64) | ones(1)]                 at cols p*193 + [0, 65)
#   B block: [zeros(32) | ones(1) | zeros(31) | vh_B(64)] at cols p*193 + [65, 193)
#   (B ones at col +97 so B sums land on psum partition 32 -- DVE start
#   partitions must be in {0, 32, 64, 96})
VHA_W = 386


def build_nc():
    # Bacc (not plain Bass): its compile() pipeline splits multi-semaphore
    # waits into event-semaphore chains -- walrus codegen allows only ONE
    # sync wait per compute instruction on TRN2.
    nc = bacc.Bacc("TRN2", target_bir_lowering=False)

    xq_d = nc.declare_dram_parameter("xq", [CT, 128, 2048], BF16, isOutput=False)
    xk_d = nc.declare_dram_parameter("xk", [CT, 128, 2048], BF16, isOutput=False)
    xv_d = nc.declare_dram_parameter("xv", [CT, 128, 2048], BF16, isOutput=False)
    wq_d = nc.declare_dram_parameter("wq", [CT, 128, 256], BF16, isOutput=False)
    wk_d = nc.declare_dram_parameter("wk", [CT, 128, 256], BF16, isOutput=False)
    wv_d = nc.declare_dram_parameter("wv", [CT, 128, 256], BF16, isOutput=False)
    wo_d = nc.declare_dram_parameter("wo", [2, 128, 1024], BF16, isOutput=False)
    bq_d = nc.declare_dram_parameter("bq2", [2, 128, 1], F32, isOutput=False)
    bk_d = nc.declare_dram_parameter("bk2", [2, 128, 1], F32, isOutput=False)
    bvb_d = nc.declare_dram_parameter("bvb", [128, 256], BF16, isOutput=False)
    mk_d = nc.declare_dram_parameter("mk", [QT, KG, 128, 2048], BF16, isOutput=False)
    yt_d = nc.declare_dram_parameter("yt", [8, 2, 128, 1024], BF16, isOutput=True)
    rscr_d = nc.dram_tensor("rscr", [QT, HPC, 512], BF16)

    with tile.TileContext(nc) as tc:
        with tc.tile_pool(name="persist", bufs=1) as singles:
            # PE warmup filler source -- first gpsimd op so it's ready fast
            warm = singles.tile([128, 512], BF16, tag="warm", name="warm")
            nc.gpsimd.memset(warm[:, :], 0.0)

            # ---- small tensors first on the DMA queues ----
            bq_sb, bk_sb = [], []
            for m in range(2):
                tq = singles.tile([128, 1], F32, tag=f"bq{m}", name=f"bq{m}")
                nc.sync.dma_start(out=tq, in_=bq_d[m])
                bq_sb.append(tq)
                tk = singles.tile([128, 1], F32, tag=f"bk{m}", name=f"bk{m}")
                nc.sync.dma_start(out=tk, in_=bk_d[m])
                bk_sb.append(tk)
            bvb_sb = singles.tile([128, 256], BF16, tag="bvb", name="bvb")
            nc.sync.dma_start(out=bvb_sb, in_=bvb_d[:, :])

            def load_rows(dram, n_tiles, width, tag):
                tiles = []
                for c in range(n_tiles):
                    t = singles.tile([128, width], BF16, tag=f"{tag}{c}", name=f"{tag}{c}")
                    nc.sync.dma_start(out=t, in_=dram[c])
                    tiles.append(t)
                return tiles

            wk_sb = load_rows(wk_d, CT, 256, "wk")
            wv_sb = load_rows(wv_d, CT, 256, "wv")
            wq_sb = load_rows(wq_d, CT, 256, "wq")
            wo_sb = load_rows(wo_d, 2, DIM, "wo")

            # ---- x tensors, wide rows, priority order ----
            def alloc_x(tag):
                return [singles.tile([128, 2048], BF16, tag=f"{tag}{c}",
                                     name=f"{tag}{c}") for c in range(CT)]

            xk_sb = alloc_x("xk")
            for c in range(CT):
                nc.sync.dma_start(out=xk_sb[c], in_=xk_d[c])
            xv_sb = alloc_x("xv")
            for c in range(CT):
                nc.sync.dma_start(out=xv_sb[c], in_=xv_d[c])
            xq_sb = alloc_x("xq")
            for c in range(CT):  # n=0 columns first (unblocks attention)
                nc.sync.dma_start(out=xq_sb[c][:, 0:512], in_=xq_d[c][:, 0:512])

            # first two mask groups before the xq tail
            mask_tiles = {}

            def load_mask(qt, g):
                t = singles.tile([128, 2048], BF16, tag="mask", name="mask",
                                 bufs=3)
                nc.sync.dma_start(out=t, in_=mk_d[qt, g])
                mask_tiles[(qt, g)] = t

            load_mask(0, 0)
            load_mask(0, 1)

            for c in range(CT):  # xq n=1..3 columns
                nc.sync.dma_start(out=xq_sb[c][:, 512:2048],
                                  in_=xq_d[c][:, 512:2048])

            # ---- persistent intermediates ----
            qhT = [[singles.tile([128, 512], BF16, tag=f"qhT{m}_{n}",
                                 name=f"qhT{m}_{n}") for n in range(QT)]
                   for m in range(2)]
            khT = [[singles.tile([128, 512], BF16, tag=f"khT{m}_{n}",
                                 name=f"khT{m}_{n}") for n in range(QT)]
                   for m in range(2)]
            OT = [singles.tile([128, S], BF16, tag=f"OT{m}", name=f"OT{m}") for m in range(2)]
            vha = [singles.tile([128, VHA_W], BF16, tag=f"vha{kt}",
                                name=f"vha{kt}") for kt in range(KT)]
            sums_stage = singles.tile([128, 2, S], F32, tag="sums_stage",
                                      name="sums_stage")

            for kt in range(KT):
                for p in range(2):
                    base = p * 193
                    nc.gpsimd.memset(vha[kt][:, base + 64:base + 65], 1.0)
                    nc.gpsimd.memset(vha[kt][:, base + 97:base + 98], 1.0)
                    nc.gpsimd.memset(vha[kt][:, base + 65:base + 97], 0.0)
                    nc.gpsimd.memset(vha[kt][:, base + 98:base + 129], 0.0)

            def bias_bc(b_sb, n):
                bb = b_sb[:, 0:1]
                return bass.AP(tensor=bb.tensor, offset=bb.offset,
                               ap=[list(bb.ap[0]), [0, n]])

            # ---- K projection: c-outer (DMA-arrival-paced), 8 psum banks ----
            with tc.tile_pool(name="pjk", bufs=1, space="PSUM") as pjk:
                kps = [[pjk.tile([128, 512], F32, tag=f"k{m}{n}",
                                 name=f"kps{m}{n}") for n in range(QT)]
                       for m in range(2)]
                # warmup: open the HAM clock gate while DMAs land
                for i in range(10):
                    m, n = (i % 8) // 4, i % 4
                    nc.tensor.matmul(kps[m][n], warm[:, 0:128], warm[:, :],
                                     start=True, stop=True)
                for c in range(CT):
                    for m in range(2):
                        for n in range(QT):
                            nc.tensor.matmul(
                                kps[m][n],
                                wk_sb[c][:, m * 128:(m + 1) * 128],
                                xk_sb[c][:, n * 512:(n + 1) * 512],
                                start=(c == 0), stop=(c == CT - 1))
                for m in range(2):
                    for n in range(QT):
                        nc.vector.tensor_tensor(
                            out=khT[m][n], in0=kps[m][n],
                            in1=bias_bc(bk_sb[m], 512),
                            op=mybir.AluOpType.add)

            # ---- V projection: c-outer, two waves of 8 kt (1 bank per kt:
            # a psum bank can hold only ONE accumulation group at a time) ----
            with tc.tile_pool(name="pjv", bufs=1, space="PSUM") as pjv:
                for wave in range(2):
                    k0 = wave * 8
                    vps = [pjv.tile([128, 256], F32, tag=f"v{i}",
                                    name=f"vps{i}") for i in range(8)]
                    for c in range(CT):
                        for i in range(8):
                            kt = k0 + i
                            nc.tensor.matmul(
                                vps[i],
                                xv_sb[c][:, kt * 128:(kt + 1) * 128],
                                wv_sb[c],
                                start=(c == 0), stop=(c == CT - 1))
                    for i in range(8):
                        kt = k0 + i
                        for h in range(HPC):
                            p, is_b = h // 2, h % 2
                            col = p * 193 + (129 if is_b else 0)
                            nc.vector.tensor_tensor(
                                out=vha[kt][:, col:col + 64],
                                in0=vps[i][:, h * 64:(h + 1) * 64],
                                in1=bvb_sb[:, h * 64:(h + 1) * 64],
                                op=mybir.AluOpType.add)

            # ---- attention (+ Q-proj n=0 upfront, n+1 at each qt boundary) ----
            with tc.tile_pool(name="scp", bufs=2, space="PSUM") as scp, \
                 tc.tile_pool(name="pvp", bufs=2, space="PSUM") as pvp:

                def q_proj(n, m_list):
                    qps = scp.tile([128, 1024], F32, tag="sc", name="qps")
                    for m in m_list:
                        for c in range(CT):
                            nc.tensor.matmul(
                                qps[:, m * 512:(m + 1) * 512],
                                wq_sb[c][:, m * 128:(m + 1) * 128],
                                xq_sb[c][:, n * 512:(n + 1) * 512],
                                start=(c == 0), stop=(c == CT - 1))
                        nc.vector.tensor_tensor(
                            out=qhT[m][n], in0=qps[:, m * 512:(m + 1) * 512],
                            in1=bias_bc(bq_sb[m], 512),
                            op=mybir.AluOpType.add)

                q_proj(0, [0, 1])

                for qt in range(QT):
                    po = [pvp.tile([128, 1024], F32, tag="po", name="po")
                          for _ in range(2)]
                    for kt in range(KT):
                        g, kl = kt // KG, kt % KG
                        if (qt, g) not in mask_tiles:
                            load_mask(qt, g)
                        mt = mask_tiles[(qt, g)]
                        m_ap = mt[:, kl * 512:(kl + 1) * 512]
                        mbc = bass.AP(
                            tensor=m_ap.tensor,
                            offset=m_ap.offset,
                            ap=[list(m_ap.ap[0]), [0, 2], list(m_ap.ap[1])])
                        for p in range(2):
                            ps = scp.tile([128, 1024], F32, tag="sc", name="ps")
                            for ab in range(2):
                                nc.tensor.matmul(
                                    ps[:, ab * 512:(ab + 1) * 512],
                                    khT[p][kt // 4][ab * 64:(ab + 1) * 64,
                                                    (kt % 4) * 128:(kt % 4 + 1) * 128],
                                    qhT[p][qt][ab * 64:(ab + 1) * 64, :],
                                    start=True, stop=True)
                            pt = singles.tile([128, 1024], BF16, tag="pt",
                                              name="pt", bufs=6)
                            nc.scalar.activation(
                                out=pt, in_=ps,
                                func=mybir.ActivationFunctionType.Exp,
                                scale=float(SCALE))
                            nc.vector.tensor_tensor(
                                out=pt, in0=pt, in1=mbc,
                                op=mybir.AluOpType.mult)
                            base = p * 193
                            nc.tensor.matmul(
                                po[p][0:65, 0:512],
                                vha[kt][:, base:base + 65],
                                pt[:, 0:512],
                                start=(kt == 0), stop=(kt == KT - 1))
                            nc.tensor.matmul(
                                po[p][:, 512:1024],
                                vha[kt][:, base + 65:base + 193],
                                pt[:, 512:1024],
                                start=(kt == 0), stop=(kt == KT - 1))
                        del mt
                        # next qt's q-projection, emitted where PE has slack
                        if qt < QT - 1 and kt == 11:
                            q_proj(qt + 1, [0])
                        if qt < QT - 1 and kt == 13:
                            q_proj(qt + 1, [1])

                    # ---- per-qt: evacuate po, compute 1/sums, normalize ----
                    qsl = slice(qt * 512, (qt + 1) * 512)
                    for p in range(2):
                        nc.vector.tensor_copy(
                            out=OT[p][0:64, qsl], in_=po[p][0:64, 0:512])
                        nc.vector.tensor_copy(
                            out=OT[p][64:128, qsl], in_=po[p][64:128, 512:1024])
                        nc.vector.tensor_copy(
                            out=sums_stage[64:65, p, qsl],
                            in_=po[p][64:65, 0:512])
                        nc.vector.tensor_copy(
                            out=sums_stage[32:33, p, qsl],
                            in_=po[p][32:33, 512:1024])
                    recin = singles.tile([128, 16], F32, tag="recin",
                                         name="recin", bufs=2)
                    for h in range(HPC):
                        p, is_b = h // 2, h % 2
                        row = 32 if is_b else 64
                        nc.sync.dma_start(
                            out=recin[:, h * 4:(h + 1) * 4],
                            in_=sums_stage[row:row + 1, p, qsl])
                    recout = singles.tile([128, 16], F32, tag="recout",
                                          name="recout", bufs=2)
                    nc.vector.reciprocal(out=recout, in_=recin)
                    recout_bf = singles.tile([128, 16], BF16, tag="recout_bf",
                                             name="recout_bf", bufs=2)
                    nc.vector.tensor_copy(out=recout_bf, in_=recout)
                    for h in range(HPC):
                        nc.sync.dma_start(
                            out=rscr_d[qt, h],
                            in_=recout_bf[:, h * 4:(h + 1) * 4])
                    for p in range(2):
                        rbc = singles.tile([128, 512], BF16, tag="rbc",
                                           name="rbc", bufs=4)
                        for ab in range(2):
                            srow = rscr_d[qt, 2 * p + ab]
                            src_bc = bass.AP(
                                tensor=srow.tensor, offset=srow.offset,
                                ap=[[0, 64], list(srow.ap[-1])])
                            nc.sync.dma_start(
                                out=rbc[ab * 64:(ab + 1) * 64, :], in_=src_bc)
                        nc.vector.tensor_tensor(
                            out=OT[p][:, qsl], in0=OT[p][:, qsl], in1=rbc,
                            op=mybir.AluOpType.mult)

            # ---- output projection ----
            with tc.tile_pool(name="oyp", bufs=4, space="PSUM") as oyp:
                idx = 0
                for ot in range(8):
                    for half in range(2):
                        ps = oyp.tile([128, 1024], F32, tag="py", name="psy")
                        for p in range(2):
                            for n2 in range(2):
                                nc.tensor.matmul(
                                    ps[:, n2 * 512:(n2 + 1) * 512],
                                    wo_sb[p][:, ot * 128:(ot + 1) * 128],
                                    OT[p][:, (half * 2 + n2) * 512:
                                          (half * 2 + n2 + 1) * 512],
                                    start=(p == 0), stop=(p == 1))
                        yt = singles.tile([128, 1024], BF16, tag="yt",
                                          name="yt", bufs=4)
                        # split psum->sbuf casts across DVE and ACT at the tail
                        if idx % 8 < 5:
                            nc.vector.tensor_copy(out=yt, in_=ps)
                        else:
                            nc.scalar.copy(out=yt, in_=ps)
                        idx += 1
                        nc.sync.dma_start(out=yt_d[ot, half], in_=yt)
    nc.compile()
    return nc


_NC_CACHE = None


def get_nc():
    global _NC_CACHE
    if _NC_CACHE is None:
        _NC_CACHE = build_nc()
    return _NC_CACHE


def prep_in_maps(q, k, v, mask, Wq, bq, Wk, bk, Wv, bv, Wo, bo):
    q = np.asarray(q, np.float32)
    k = np.asarray(k, np.float32)
    v = np.asarray(v, np.float32)
    mask = np.asarray(mask)
    WqT = np.asarray(Wq, np.float32).T
    WkT = np.asarray(Wk, np.float32).T
    WvT = np.asarray(Wv, np.float32).T
    WoT = np.asarray(Wo, np.float32).T
    bq = np.asarray(bq, np.float32)
    bk = np.asarray(bk, np.float32)
    bv = np.asarray(bv, np.float32)

    xT = {}
    keepT = {}
    for b in range(B):
        xT[b] = tuple(
            np.ascontiguousarray(arr[b].T).astype(NP_BF16).reshape(CT, 128, 2048)
            for arr in (q, k, v))
        mt = np.ascontiguousarray((~mask[b, 0]).T.astype(np.float32)).astype(NP_BF16)
        # [kv, q] -> [KT, QT, 128, 512] -> [QT, KG, 128, KG*512] kt-interleaved
        t = mt.reshape(KT, 128, QT, 512).transpose(0, 2, 1, 3)
        keepT[b] = np.ascontiguousarray(
            t.reshape(KT // KG, KG, QT, 128, 512)
            .transpose(2, 0, 3, 1, 4).reshape(QT, KG, 128, KG * 512))

    in_maps = []
    for c in range(N_CORES):
        b = c // 4
        ho = c % 4
        dsl = slice(ho * 256, ho * 256 + 256)
        xq, xk, xv = xT[b]
        in_maps.append({
            "xq": xq,
            "xk": xk,
            "xv": xv,
            "wq": np.ascontiguousarray(WqT[:, dsl]).astype(NP_BF16).reshape(CT, 128, 256),
            "wk": np.ascontiguousarray(WkT[:, dsl]).astype(NP_BF16).reshape(CT, 128, 256),
            "wv": np.ascontiguousarray(WvT[:, dsl]).astype(NP_BF16).reshape(CT, 128, 256),
            "wo": np.ascontiguousarray(WoT[dsl, :]).astype(NP_BF16).reshape(2, 128, 1024),
            "bq2": np.ascontiguousarray(bq[dsl]).reshape(2, 128, 1).astype(np.float32),
            "bk2": np.ascontiguousarray(bk[dsl]).reshape(2, 128, 1).astype(np.float32),
            "bvb": np.ascontiguousarray(
                np.broadcast_to(bv[dsl], (128, 256))).astype(NP_BF16),
            "mk": keepT[b],
        })
    return in_maps


def gather_output(results, bo):
    bo = np.asarray(bo, np.float32)
    y = np.zeros((B, S, DIM), np.float32)
    for c in range(N_CORES):
        yt = np.asarray(results[c]["yt"], np.float32)  # [8, 2, 128, 1024]
        yT = yt.transpose(0, 2, 1, 3).reshape(DIM, S)
        y[c // 4] += yT.T
    y += bo[None, None, :]
    return y


def kernel(**inputs):
    nc = get_nc()
    in_maps = prep_in_maps(**{k_: inputs[k_] for k_ in (
        "q", "k", "v", "mask", "Wq", "bq", "Wk", "bk", "Wv", "bv", "Wo", "bo")})
    res = bass_utils.run_bass_kernel_spmd(nc, in_maps, list(range(N_CORES)))
    return gather_output(res.results, inputs["bo"])
